# revision 17
# baseline (speedup 1.0000x reference)
"""Trainium2 Bass kernel for nn_MF_9612136808809 (matrix-factorization loss).

Reference computation:
    sel        = user_emb[center_uid]                  # [B, C]
    pref_score = sel @ item_emb.T                      # [B, ITEM_NUM]
    mask[r, seq[r, k]] = True                          # unique (row, item) pairs
    loss = sum(where(mask, (s-1)^2, s^2))
returns (loss, pref_score).

Strategy (8 NeuronCores, tensor-parallel over the item dim):
  - Each core owns a 12500-column shard of item_emb / pref_score and streams
    selT.T @ item_embT through PSUM into DRAM (memory-bound: ~51 MB of score
    writes per core dominates).
  - loss is decomposed exactly as  sum_all s^2 + sum_pos (1 - 2 s):
      * sum_all s^2 = sum_r sel_r (E^T E) sel_r^T with the 64x64 Gram matrix
        E^T E accumulated on the PE over the core's item shard - avoids any
        per-score elementwise work.
      * sum_pos: sharded over batch rows (128 rows/core). Item embeddings for
        seq are gathered on-device; s_pos = <sel_r, item_emb[seq[r,k]]> with
        host-computed first-occurrence weights deduplicating repeated items.
  - Per-core partial losses are summed on the host (final loss all-reduce).

Written in raw Bass (explicit semaphores): the walrus build in this container
rejects TileContext's epilogue (multi-wait Drain, RANGE_CLEAR encoding), and
Matmult supports only one attached semaphore wait - raw Bass uses standalone
wait instructions, which sidesteps both.
"""

import sys
from contextlib import ExitStack

for _p in ("/opt/trn_rl_repo", "/root/.axon_site/_ro/trn_rl_repo"):
    if _p not in sys.path:
        sys.path.append(_p)

import numpy as np

import concourse.bass as bass
import concourse.mybir as mybir

F32 = mybir.dt.float32
I32 = mybir.dt.int32

# Problem constants (hardcoded per harness contract).
B, HIST, CH = 1024, 50, 64
USER_NUM, ITEM_NUM = 100000, 100000
NCORES = 8
SHARD = ITEM_NUM // NCORES  # 12500
P = 128


def build_nc(
    user_num=USER_NUM,
    item_num=ITEM_NUM,
    shard=SHARD,
    b=B,
    hist=HIST,
    ch=CH,
    nch=500,   # matmul free-dim chunk (<=512 f32, one PSUM bank)
    och=2500,  # output-tile columns (multiple of nch; one DMA each)
    obufs=3,   # output SBUF tiles in rotation
    debug=False,
):
    """Build the per-core SPMD Bass program. All per-core variation is data."""
    assert b % P == 0 and shard % och == 0 and och % nch == 0
    M = b // P            # row chunks (8)
    NN = shard // nch     # matmul chunks per row chunk (25)
    NO = shard // och     # output tiles per row chunk (5)
    OCH_N = och // nch    # matmul chunks per output tile (5)
    D = M * NO            # total output DMAs (40)
    R = 7                 # rotating score PSUM banks
    GFULL = shard // P    # full gram chunks (97)
    GTAIL = shard % P     # tail gram chunk rows (84)
    GN = GFULL + (1 if GTAIL else 0)
    mult, add = mybir.AluOpType.mult, mybir.AluOpType.add
    X = mybir.AxisListType.X

    nc = bass.Bass()

    user_emb = nc.declare_dram_parameter("user_emb", [user_num, ch], F32, False)
    item_emb = nc.declare_dram_parameter("item_emb", [item_num, ch], F32, False)
    item_t = nc.declare_dram_parameter("item_t", [ch, shard], F32, False)
    item_s = nc.declare_dram_parameter("item_s", [shard, ch], F32, False)
    uid_t = nc.declare_dram_parameter("uid_t", [P, M], I32, False)
    uid_c = nc.declare_dram_parameter("uid_c", [P, 1], I32, False)
    seq_c = nc.declare_dram_parameter("seq_c", [P, hist], I32, False)
    w_c = nc.declare_dram_parameter("w_c", [P, hist], F32, False)
    score = nc.declare_dram_parameter("score", [b, shard], F32, True)
    loss_part = nc.declare_dram_parameter("loss_part", [1, 1], F32, True)
    if debug:
        GNq = GFULL + (1 if GTAIL else 0)
        d_sel = nc.declare_dram_parameter("d_sel", [P, M, ch], F32, True)
        d_selT = nc.declare_dram_parameter("d_selT", [ch, b], F32, True)
        d_gpos = nc.declare_dram_parameter("d_gpos", [P, hist, ch], F32, True)
        d_gall = nc.declare_dram_parameter("d_gall", [P, GNq, ch], F32, True)
        d_gsb = nc.declare_dram_parameter("d_gsb", [ch, ch], F32, True)
        d_acc = nc.declare_dram_parameter("d_acc", [P, M + 2], F32, True)
        d_spos = nc.declare_dram_parameter("d_spos", [P, hist], F32, True)
        d_red = nc.declare_dram_parameter("d_red", [P, 1], F32, True)

    with ExitStack() as ctx:
        ec = ctx.enter_context
        # --- SBUF ---
        identity = ec(nc.sbuf_tensor("identity", [P, P], F32))
        ones = ec(nc.sbuf_tensor("ones", [P, 1], F32))
        itemT = ec(nc.sbuf_tensor("itemT", [ch, shard], F32))
        uidt = ec(nc.sbuf_tensor("uidt", [P, M], I32))
        uidc = ec(nc.sbuf_tensor("uidc", [P, 1], I32))
        seqs = ec(nc.sbuf_tensor("seqs", [P, hist], I32))
        ws = ec(nc.sbuf_tensor("ws", [P, hist], F32))
        sel = ec(nc.sbuf_tensor("sel", [P, M, ch], F32))
        selT = ec(nc.sbuf_tensor("selT", [ch, b], F32))
        selc = ec(nc.sbuf_tensor("selc", [P, ch], F32))
        gpos = ec(nc.sbuf_tensor("gpos", [P, hist, ch], F32))
        gall = ec(nc.sbuf_tensor("gall", [P, GN, ch], F32))
        g_sb = ec(nc.sbuf_tensor("g_sb", [ch, ch], F32))
        outs = [ec(nc.sbuf_tensor(f"out{i}", [P, och], F32)) for i in range(obufs)]
        pscr = ec(nc.sbuf_tensor("pscr", [P, ch], F32))
        wscr = ec(nc.sbuf_tensor("wscr", [P, hist], F32))
        spos = ec(nc.sbuf_tensor("spos", [P, hist], F32))
        acc = ec(nc.sbuf_tensor("acc", [P, M + 2], F32))
        zscr = ec(nc.sbuf_tensor("zscr", [P, ch], F32))
        red = ec(nc.sbuf_tensor("red", [P, 1], F32))
        lsb = ec(nc.sbuf_tensor("lsb", [1, 1], F32))

        # --- PSUM: 7 rotating score banks + 1 misc bank (G | Z | loss) ---
        ps = [ec(nc.psum_tensor(f"ps{i}", [P, nch], F32)) for i in range(R)]
        pz = ec(nc.psum_tensor("pz", [P, P], F32))

        # --- semaphores ---
        s_it = ec(nc.semaphore("s_it"))       # itemT load
        s_lds = ec(nc.semaphore("s_lds"))     # 4 small loads
        s_sel = ec(nc.semaphore("s_sel"))     # sel gather
        s_selc = ec(nc.semaphore("s_selc"))   # selc gather
        s_gpos = ec(nc.semaphore("s_gpos"))   # gpos gather
        s_gall = ec(nc.semaphore("s_gall"))   # gram chunk loads
        s_gp = ec(nc.semaphore("s_gp"))       # gpsimd compute (identity/ones)
        s_pe = ec(nc.semaphore("s_pe"))       # PE instruction counter
        s_act = ec(nc.semaphore("s_act"))     # ACT instruction counter
        s_dv2 = ec(nc.semaphore("s_dv2"))     # DVE op counter (ordering chain)
        s_ob = [ec(nc.semaphore(f"s_ob{i}")) for i in range(obufs)]  # out DMAs
        s_loss = ec(nc.semaphore("s_loss"))
        dbg_sem = [ec(nc.semaphore("s_dbg"))] if debug else [None]

        gall_dmas = GN

        with nc.Block() as block:

            @block.gpsimd
            def _(gp):
                # identity + ones (s_gp: 1..3)
                gp.memset(identity[:, :], 0.0).then_inc(s_gp, 1)
                # gpsimd fans out over parallel Q7 cores: same-engine RAW on
                # `identity` still needs a semaphore
                gp.wait_ge(s_gp, 1)
                gp.affine_select(
                    out=identity[:, :],
                    in_=identity[:, :],
                    compare_op=mybir.AluOpType.not_equal,
                    fill=1.0,
                    base=0,
                    pattern=[[-1, P]],
                    channel_multiplier=1,
                ).then_inc(s_gp, 1)
                gp.memset(ones[:, :], 1.0).then_inc(s_gp, 1)
                # gathers (need index tensors)
                gp.wait_ge(s_lds, 64)
                for m in range(M):
                    gp.indirect_dma_start(
                        out=sel[:, m, :],
                        out_offset=None,
                        in_=user_emb[:, :],
                        in_offset=bass.IndirectOffsetOnAxis(
                            ap=uidt[:, m : m + 1], axis=0
                        ),
                    ).then_inc(s_sel, 16)
                gp.indirect_dma_start(
                    out=selc[:, :],
                    out_offset=None,
                    in_=user_emb[:, :],
                    in_offset=bass.IndirectOffsetOnAxis(ap=uidc[:, :1], axis=0),
                ).then_inc(s_selc, 16)
                for k in range(hist):
                    gp.indirect_dma_start(
                        out=gpos[:, k, :],
                        out_offset=None,
                        in_=item_emb[:, :],
                        in_offset=bass.IndirectOffsetOnAxis(
                            ap=seqs[:, k : k + 1], axis=0
                        ),
                    ).then_inc(s_gpos, 16)
                # gram chunk loads: item_s rows -> [P, GN, ch] (SWDGE; overlaps
                # the HWDGE output stream)
                for g in range(GFULL):
                    gp.dma_start(
                        out=gall[:, g, :], in_=item_s[g * P : (g + 1) * P, :]
                    ).then_inc(s_gall, 16)
                if GTAIL:
                    gp.dma_start(
                        out=gall[:GTAIL, GFULL, :], in_=item_s[GFULL * P :, :]
                    ).then_inc(s_gall, 16)

            @block.sync
            def _(sy):
                sy.dma_start(out=itemT[:, :], in_=item_t[:, :]).then_inc(s_it, 16)
                sy.dma_start(out=uidt[:, :], in_=uid_t[:, :]).then_inc(s_lds, 16)
                sy.dma_start(out=uidc[:, :], in_=uid_c[:, :]).then_inc(s_lds, 16)
                sy.dma_start(out=seqs[:, :], in_=seq_c[:, :]).then_inc(s_lds, 16)
                sy.dma_start(out=ws[:, :], in_=w_c[:, :]).then_inc(s_lds, 16)
                # output stream: DMA d covers rows m*P..(m+1)*P, cols no*och..
                for d in range(D):
                    m, no = divmod(d, NO)
                    # all OCH_N copies into this tile done (ACT in-order):
                    # copy c is ACT instr 8 + c + 1 -> s_act = 9 + c
                    sy.wait_ge(s_act, M + OCH_N * (d + 1))
                    sy.dma_start(
                        out=score[m * P : (m + 1) * P, no * och : (no + 1) * och],
                        in_=outs[d % obufs][:, :],
                    ).then_inc(s_ob[d % obufs], 16)
                # loss scalar
                sy.wait_ge(s_act, M + OCH_N * D + 2)
                sy.dma_start(out=loss_part[:, :], in_=lsb[:, :]).then_inc(s_loss, 16)
                if debug:
                    s_dbg = dbg_sem[0]
                    pos_ops_dbg = 2 * hist + 4
                    sy.wait_ge(s_pe, M + M * NN + GN + M + 1)
                    sy.wait_ge(s_dv2, pos_ops_dbg + 2 * M + 1)
                    sy.wait_ge(s_act, M + OCH_N * D + 2)
                    for dst_ap, src_ap in [
                        (d_sel[:, :, :], sel[:, :, :]),
                        (d_selT[:, :], selT[:, :]),
                        (d_gpos[:, :, :], gpos[:, :, :]),
                        (d_gall[:, :, :], gall[:, :, :]),
                        (d_gsb[:, :], g_sb[:, :]),
                        (d_acc[:, :], acc[:, :]),
                        (d_spos[:, :], spos[:, :]),
                        (d_red[:, :], red[:, :]),
                    ]:
                        sy.dma_start(out=dst_ap, in_=src_ap).then_inc(s_dbg, 16)
                    sy.wait_ge(s_dbg, 16 * 8)
                # drain: every DMA completion is awaited before kernel end
                for i in range(obufs):
                    n_i = (D - i + obufs - 1) // obufs
                    sy.wait_ge(s_ob[i], 16 * n_i)
                sy.wait_ge(s_loss, 16)

            @block.tensor
            def _(pe):
                # PE stream: transposes (1..M), score (M+1..M+CK),
                # gram (+GN), Z (+M), loss (+1)
                pe.wait_ge(s_gp, 3)
                pe.wait_ge(s_sel, 16 * M)
                for m in range(M):
                    if m >= R:
                        # bank WAR: selT copy (m-R) must have read this bank
                        pe.wait_ge(s_act, m - R + 1)
                    pe.transpose(
                        out=ps[m % R][: ch, :P],
                        in_=sel[:, m, :],
                        identity=identity[:, :],
                    ).then_inc(s_pe, 1)
                pe.wait_ge(s_it, 16)
                pe.wait_ge(s_act, M)  # all selT copies done (frees all banks)
                idx = 0
                for m in range(M):
                    for n in range(NN):
                        if idx >= R:
                            # bank WAR: out-copy (idx-R) must have read it
                            pe.wait_ge(s_act, M + idx - R + 1)
                        pe.matmul(
                            out=ps[idx % R][:, :],
                            lhsT=selT[:, m * P : (m + 1) * P],
                            rhs=itemT[:, n * nch : (n + 1) * nch],
                            start=True,
                            stop=True,
                        ).then_inc(s_pe, 1)
                        idx += 1
                # Gram matrix E^T E over the item shard -> pz[:ch, :ch]
                pe.wait_ge(s_gall, 16 * gall_dmas)
                for g in range(GN):
                    kg = P if (g < GFULL) else GTAIL
                    pe.matmul(
                        out=pz[:ch, :ch],
                        lhsT=gall[:kg, g, :],
                        rhs=gall[:kg, g, :],
                        start=(g == 0),
                        stop=(g == GN - 1),
                    ).then_inc(s_pe, 1)
                # Z_m = sel_m @ G -> pz[:, ch:2ch]; ping-pong with DVE q_m
                pe.wait_ge(s_act, M + OCH_N * D + 1)  # G copied to SBUF
                pos_ops = 2 * hist + 4
                for m in range(M):
                    if m > 0:
                        pe.wait_ge(s_dv2, pos_ops + 2 * m)  # q_{m-1} done
                    pe.matmul(
                        out=pz[:, ch : 2 * ch],
                        lhsT=selT[:, m * P : (m + 1) * P],
                        rhs=g_sb[:, :],
                        start=True,
                        stop=True,
                    ).then_inc(s_pe, 1)
                # loss scalar: sum over partitions via ones-matmul
                pe.wait_ge(s_dv2, pos_ops + 2 * M + 1)  # fold done
                pe.matmul(
                    out=pz[:1, :1], lhsT=red[:, :], rhs=ones[:, :],
                    start=True, stop=True,
                ).then_inc(s_pe, 1)

            CK = M * NN

            @block.scalar
            def _(ac):
                # selT copies: ACT instrs 1..M
                for m in range(M):
                    ac.wait_ge(s_pe, m + 1)
                    ac.copy(
                        out=selT[:, m * P : (m + 1) * P],
                        in_=ps[m % R][: ch, :P],
                    ).then_inc(s_act, 1)
                # score copies: ACT instrs M+1 .. M+CK
                idx = 0
                for d in range(D):
                    for j in range(OCH_N):
                        ac.wait_ge(s_pe, M + idx + 1)
                        if j == 0 and d >= obufs:
                            # buffer WAR: DMA d-obufs on this buffer done
                            ac.wait_ge(s_ob[d % obufs], 16 * (d // obufs))
                        ac.copy(
                            out=outs[d % obufs][:, j * nch : (j + 1) * nch],
                            in_=ps[idx % R][:, :],
                        ).then_inc(s_act, 1)
                        idx += 1
                # G -> SBUF (ACT instr M+CK+1)
                ac.wait_ge(s_pe, M + CK + GN)
                ac.copy(out=g_sb[:, :], in_=pz[:ch, :ch]).then_inc(s_act, 1)
                # loss scalar -> SBUF (ACT instr M+CK+2)
                ac.wait_ge(s_pe, M + CK + GN + M + 1)
                ac.copy(out=lsb[:, :], in_=pz[:1, :1]).then_inc(s_act, 1)

            @block.vector
            def _(ve):
                # DVE-DVE data deps need an explicit sem chain (the engine is
                # in-order on HW so these waits are free, but the race
                # detector tracks ordering only through semaphores).
                dv = [0]

                def step(inst):
                    inst.then_inc(s_dv2, 1)
                    dv[0] += 1
                    return inst

                def chain():
                    if dv[0]:
                        ve.wait_ge(s_dv2, dv[0])

                # positives: s_pos[r,k] = <sel_r, item_emb[seq[r,k]]>
                ve.wait_ge(s_selc, 16)
                ve.wait_ge(s_gpos, 16 * hist)
                ve.wait_ge(s_lds, 64)
                for k in range(hist):
                    chain()
                    step(ve.tensor_tensor(
                        out=pscr[:, :], in0=gpos[:, k, :], in1=selc[:, :], op=mult
                    ))
                    chain()
                    step(ve.tensor_reduce(
                        out=spos[:, k : k + 1], in_=pscr[:, :], axis=X, op=add
                    ))
                # acc[:, M] = -2 * sum_k w * s_pos ; acc[:, M+1] = sum_k w
                chain()
                step(ve.tensor_tensor(
                    out=wscr[:, :], in0=spos[:, :], in1=ws[:, :], op=mult
                ))
                chain()
                step(ve.tensor_scalar_mul(wscr[:, :], wscr[:, :], -2.0))
                chain()
                step(ve.tensor_reduce(
                    out=acc[:, M : M + 1], in_=wscr[:, :], axis=X, op=add
                ))
                chain()
                step(ve.tensor_reduce(
                    out=acc[:, M + 1 : M + 2], in_=ws[:, :], axis=X, op=add
                ))
                # q_m = sum_c Z_m * sel_m  (sum_all s^2 partials)
                ve.wait_ge(s_sel, 16 * M)
                for m in range(M):
                    ve.wait_ge(s_pe, M + CK + GN + m + 1)
                    chain()
                    step(ve.tensor_tensor(
                        out=zscr[:, :],
                        in0=pz[:, ch : 2 * ch],
                        in1=sel[:, m, :],
                        op=mult,
                    ))
                    chain()
                    step(ve.tensor_reduce(
                        out=acc[:, m : m + 1], in_=zscr[:, :], axis=X, op=add
                    ))
                # fold all partials
                chain()
                step(ve.tensor_reduce(
                    out=red[:, :], in_=acc[:, :], axis=X, op=add
                ))

    return nc


def make_in_maps(user_emb, item_emb, center_uid, seq, ncores=NCORES, shard=SHARD):
    """Host-side sharding: per-core input maps."""
    b, hist = seq.shape
    M = b // P
    user_emb = np.ascontiguousarray(np.asarray(user_emb, dtype=np.float32))
    item_emb = np.ascontiguousarray(np.asarray(item_emb, dtype=np.float32))
    uid = np.asarray(center_uid).astype(np.int32)
    seq = np.asarray(seq).astype(np.int32)

    item_T = np.ascontiguousarray(item_emb.T)  # [CH, ITEM_NUM]
    uid_t = np.ascontiguousarray(uid.reshape(M, P).T)  # [P, M]

    # first-occurrence (dedup) weights per row
    eq = seq[:, :, None] == seq[:, None, :]  # [b, hist, hist]
    dup = (np.tril(eq, -1)).any(axis=2)      # seen earlier in the row
    w = (~dup).astype(np.float32)            # [b, hist]

    in_maps = []
    for c in range(ncores):
        rows = slice(c * P, (c + 1) * P)
        in_maps.append(
            {
                "user_emb": user_emb,
                "item_emb": item_emb,
                "item_t": np.ascontiguousarray(
                    item_T[:, c * shard : (c + 1) * shard]
                ),
                "item_s": np.ascontiguousarray(
                    item_emb[c * shard : (c + 1) * shard, :]
                ),
                "uid_t": uid_t,
                "uid_c": np.ascontiguousarray(uid[rows].reshape(P, 1)),
                "seq_c": np.ascontiguousarray(seq[rows]),
                "w_c": np.ascontiguousarray(w[rows]),
            }
        )
    return in_maps


_NC_CACHE = {}


def kernel(user_emb, item_emb, center_uid, seq):
    from concourse.bass_utils import run_bass_kernel_spmd

    if "nc" not in _NC_CACHE:
        _NC_CACHE["nc"] = build_nc()
    nc = _NC_CACHE["nc"]

    in_maps = make_in_maps(user_emb, item_emb, center_uid, seq)
    res = run_bass_kernel_spmd(nc, in_maps, core_ids=list(range(NCORES)))
    _NC_CACHE["last_results"] = res

    scores = np.empty((B, ITEM_NUM), dtype=np.float32)
    total = np.float64(0.0)
    for c in range(NCORES):
        scores[:, c * SHARD : (c + 1) * SHARD] = res.results[c]["score"]
        total += np.float64(res.results[c]["loss_part"][0, 0])
    loss = np.float32(total)
    return loss, scores
